# revision 44
# baseline (speedup 1.0000x reference)
"""MoE gate routing kernel for Trainium2 (8 NeuronCores, data-parallel over tokens).

Computes, for x[8192,7168], weight[256,7168], bias[256]:
    scores = sigmoid(x @ weight.T + bias)            # [N, 256]
    group top-2 sums over 8 groups of 32 -> pick best group
    top-8 experts within best group (global indices), weights = renormalized
    sigmoid scores * 2.5
Returns (w [8192,8] f32, idx [8192,8] i32).

Strategy: shard tokens 8-way (1024/core). The kernel is HBM-bound: fp32 x is
29.4 MB/core, and cheaper encodings of x flip router top-k decisions past the
2e-2 gate (fp16: 2.27e-2; int16-only: 2.06e-2 -- both hardware-measured, the
idx rel-err metric is dominated by a handful of group-flip tokens). So x ships
as int16 + int8 residual (24-bit fixed point, reconstruction exact to ~1e-6 =
below fp32 accumulation noise) at 3 B/elem = 22 MB, and weight ships as raw
fp32 typed f32r (7.3 MB). Accuracy is then identical to the fp32r baseline
(1.35e-2) while DMA drops 20%.

On device the reconstruction x = i16*s + i8*(s/256) runs as two passes over
otherwise-idle engines: pass 1 on ScalarE (activation copy-with-scale), pass 2
on Pool (scalar_tensor_tensor fused multiply-add) writing f32r for the
full-rate fp32r matmul. VectorE keeps the top-k chain. Work is quartered so
reconstruction pipelines against the DMA stream.

x is host-packed as [NBUF, 128, KC, 128] so each half-buffer DMA is one
contiguous descriptor per partition (full DMA rate).
"""

import sys

sys.path.insert(0, "/opt/trn_rl_repo")

from concurrent.futures import ThreadPoolExecutor

import numpy as np

import concourse.bass as bass
from concourse import bacc
import concourse.mybir as mybir
from concourse.bass_utils import run_bass_kernel_spmd
from concourse.tile import TileContext

N_CORES = 8
N_TOK = 8192
TOK_PC = N_TOK // N_CORES  # 1024 tokens per core
D = 7168
E = 256
G = 8  # groups
EPG = E // G  # 32 experts per group
TOPK = 8
ROUTE_SCALE = 2.5
KC = D // 128  # 56 k-chunks
KCH = KC // 2  # 28 k-chunks per half-buffer
KQ = KC // 4  # 14 k-chunks per recon quarter
XBUF_T = 128  # tokens per x buffer / subtile
NBUF = TOK_PC // XBUF_T  # 8 buffers/subtiles per core

f32 = mybir.dt.float32
f32r = mybir.dt.float32r
i16 = mybir.dt.int16
i8 = mybir.dt.int8
i32 = mybir.dt.int32
u32 = mybir.dt.uint32
AX = mybir.AxisListType
OP = mybir.AluOpType
ACTF = mybir.ActivationFunctionType

_cache = {}
LAST_RESULT = None  # BassKernelResults of the most recent run (for test harness)


def _build():
    nc = bacc.Bacc(None, target_bir_lowering=False)

    # x packed [NBUF, 128, KC, XBUF_T] flattened to 2D, hi/lo streams
    xh = nc.declare_dram_parameter("xh", [NBUF * 128, KC * XBUF_T], i16, isOutput=False)
    xl = nc.declare_dram_parameter("xl", [NBUF * 128, KC * XBUF_T], i8, isOutput=False)
    wT = nc.declare_dram_parameter("wT", [D, E], f32, isOutput=False)
    bias = nc.declare_dram_parameter("bias", [1, E], f32, isOutput=False)
    w_out = nc.declare_dram_parameter("w_out", [TOK_PC, TOPK], f32, isOutput=True)
    idx_out = nc.declare_dram_parameter("idx_out", [TOK_PC, TOPK], i32, isOutput=True)

    xh_v = xh.rearrange("(b p) (c n) -> b p c n", p=128, c=KC)
    xl_v = xl.rearrange("(b p) (c n) -> b p c n", p=128, c=KC)
    wT_v = wT.rearrange("(c p) e -> p c e", p=128)  # [128, KC, E]

    with TileContext(nc) as tc:
        with (
            tc.tile_pool(name="const", bufs=1) as cpool,
            tc.tile_pool(name="xh", bufs=8) as hpool,
            tc.tile_pool(name="xl", bufs=8) as lpool,
            tc.tile_pool(name="xt", bufs=3) as tpool,
            tc.tile_pool(name="x32", bufs=8) as xpool,
            tc.tile_pool(name="sb", bufs=3) as spool,
            tc.tile_pool(name="small", bufs=2) as mpool,
            tc.tile_pool(name="out", bufs=3) as opool,
            tc.tile_pool(name="psum", bufs=6, space="PSUM") as ppool,
        ):
            def dma_xpiece(s, h):
                # two quarter-DMA pairs per half: recon can start ~2us after
                # the first quarter lands instead of ~4us after the half
                out = []
                for qq in range(2):
                    q = 2 * h + qq
                    ht = hpool.tile([128, KQ, XBUF_T], i16, tag="xh")
                    nc.sync.dma_start(
                        out=ht, in_=xh_v[s, :, q * KQ : (q + 1) * KQ, :]
                    )
                    lt = lpool.tile([128, KQ, XBUF_T], i8, tag="xl")
                    nc.sync.dma_start(
                        out=lt, in_=xl_v[s, :, q * KQ : (q + 1) * KQ, :]
                    )
                    out.append((ht, lt))
                return out

            # x and weight-quarter DMAs interleave so neither stream starves:
            # x0, w0, x1, w1, w2, x2, w3, then x3..x7
            w32 = cpool.tile([128, KC, E], f32r)

            def dma_wq(q):
                sl = slice(q * KQ, (q + 1) * KQ)
                nc.sync.dma_start(out=w32[:, sl, :], in_=wT_v[:, sl, :].bitcast(f32r))

            pre = {0: dma_xpiece(0, 0) + dma_xpiece(0, 1)}
            bias_sb = cpool.tile([1, E], f32)
            nc.sync.dma_start(out=bias_sb, in_=bias[:, :])
            ones_sb = cpool.tile([1, 128], f32)
            nc.vector.memset(ones_sb, 1.0)

            dma_wq(0)
            pre[1] = dma_xpiece(1, 0) + dma_xpiece(1, 1)
            dma_wq(1)
            pre[2] = dma_xpiece(2, 0) + dma_xpiece(2, 1)
            dma_wq(2)
            dma_wq(3)

            # accumulate outputs in SBUF; a single DMA pair at the very end
            # keeps the SP sequencer's DMA stream free of data-dependent
            # waits (a per-subtile output DMA would head-of-line-block the
            # later x-input DMAs behind it)
            w8_all = cpool.tile([128, NBUF, TOPK], f32)
            idx_all = cpool.tile([128, NBUF, TOPK], u32)

            def recon_quarter(ht, lt, q, on_pool):
                """x/s = i16 + i8/256 (the x scale is folded into the host-
                scaled weights). ACT casts the hi stream to f32; the lo stream
                fuses in via DVE scalar_tensor_tensor, except one quarter per
                buffer routed to Pool (tensor_scalar+tensor_add pair) to keep
                DVE off the critical cadence. Quarter-sized tiles so matmuls
                start as soon as each quarter is reconstructed."""
                xt = tpool.tile([128, KQ, XBUF_T], f32, tag="xt")
                nc.scalar.mul(xt, ht, 1.0)
                x32q = xpool.tile([128, KQ, XBUF_T], f32r, tag="x32")
                if on_pool:
                    lo = tpool.tile([128, KQ, XBUF_T], f32, tag="lo")
                    nc.gpsimd.tensor_scalar(lo, lt, 0.00390625, None, op0=OP.mult)
                    nc.gpsimd.tensor_add(x32q, lo, xt)
                else:
                    nc.vector.scalar_tensor_tensor(
                        out=x32q,
                        in0=lt,
                        scalar=0.00390625,
                        in1=xt,
                        op0=OP.mult,
                        op1=OP.add,
                    )
                return x32q

            for s in range(NBUF):
                if s in pre:
                    pieces = pre[s]
                else:
                    pieces = dma_xpiece(s, 0) + dma_xpiece(s, 1)

                t0 = s * XBUF_T
                ps = ppool.tile([128, E], f32, tag="ps")
                # bias preload: ps[t, e] = 1 * bias[e] (plain f32 matmul)
                nc.tensor.matmul(
                    out=ps, lhsT=ones_sb, rhs=bias_sb, start=True, stop=False
                )
                for qg in range(4):
                    ht, lt = pieces[qg]
                    on_pool = qg == 3
                    x32q = recon_quarter(ht, lt, qg, on_pool)
                    if True:
                        for c in range(KQ):
                            cc = qg * KQ + c
                            nc.tensor.matmul(
                                out=ps,
                                lhsT=x32q[:, c, :],
                                rhs=w32[:, cc, :],
                                start=False,
                                stop=(cc == KC - 1),
                            )

                # sigmoid is monotone: group top-2 runs on the raw logits in
                # PSUM (DVE) in parallel with the big sigmoid (ACT), and only
                # the two per-group winners go through a tiny sigmoid after.
                # Values are bit-identical to sigmoid-then-max.
                sig = spool.tile([128, G, EPG], f32, tag="sig")
                nc.scalar.activation(
                    out=sig.rearrange("p g e -> p (g e)"), in_=ps, func=ACTF.Sigmoid
                )
                sig_flat = sig.rearrange("p g e -> p (g e)")

                ps_g = ps.rearrange("p (g e) -> p g e", g=G)
                m12L = mpool.tile([128, 2, G], f32, tag="m12L")
                nc.vector.tensor_reduce(out=m12L[:, 0, :], in_=ps_g, axis=AX.X, op=OP.max)
                scr = spool.tile([128, G, EPG], f32, tag="scr")
                nc.vector.match_replace(
                    out=scr.rearrange("p g e -> p (g e)"),
                    in_to_replace=m12L[:, 0, :],
                    in_values=ps,
                    imm_value=-1e30,
                )
                nc.vector.tensor_reduce(out=m12L[:, 1, :], in_=scr, axis=AX.X, op=OP.max)
                m12 = mpool.tile([128, 2, G], f32, tag="m12")
                nc.scalar.activation(
                    out=m12.rearrange("p a g -> p (a g)"),
                    in_=m12L.rearrange("p a g -> p (a g)"),
                    func=ACTF.Sigmoid,
                )
                gs = mpool.tile([128, G], f32, tag="gs")
                nc.vector.tensor_add(gs, m12[:, 0, :], m12[:, 1, :])  # m1 + m2

                # one-hot of best group -> multiplicative mask
                gmax = mpool.tile([128, 1], f32, tag="gmax")
                nc.vector.tensor_reduce(out=gmax, in_=gs, axis=AX.X, op=OP.max)
                eq = mpool.tile([128, G, 1], f32, tag="eq")
                nc.vector.tensor_scalar(eq[:, :, 0], gs, gmax, None, op0=OP.is_ge)
                # masked scores: kept group unchanged (x1.0), others -> 0.0
                masked = spool.tile([128, G, EPG], f32, tag="masked")
                ba, bb = bass.broadcast_tensor_aps(sig[:, :, :], eq[:, :, :])
                nc.vector.tensor_tensor(out=masked, in0=ba, in1=bb, op=OP.mult)
                masked_flat = masked.rearrange("p g e -> p (g e)")

                vals8 = mpool.tile([128, TOPK], f32, tag="vals8")
                nc.vector.max(out=vals8, in_=masked_flat)
                nc.vector.max_index(
                    out=idx_all[:, s, :], in_max=vals8, in_values=masked_flat
                )

                ssum = mpool.tile([128, 1], f32, tag="ssum")
                nc.vector.tensor_reduce(out=ssum, in_=vals8, axis=AX.X, op=OP.add)
                rcp = mpool.tile([128, 1], f32, tag="rcp")
                nc.vector.reciprocal(out=rcp, in_=ssum)
                nc.vector.tensor_scalar(
                    w8_all[:, s, :], vals8, rcp, ROUTE_SCALE, op0=OP.mult, op1=OP.mult
                )

            w_out_v = w_out.rearrange("(s p) k -> p s k", p=128)
            idx_out_v = idx_out.rearrange("(s p) k -> p s k", p=128)
            nc.sync.dma_start(
                out=idx_out_v[:, : NBUF - 1, :],
                in_=idx_all[:, : NBUF - 1, :].bitcast(i32),
            )
            nc.sync.dma_start(
                out=w_out_v[:, : NBUF - 1, :], in_=w8_all[:, : NBUF - 1, :]
            )
            nc.sync.dma_start(
                out=idx_out_v[:, NBUF - 1 :, :],
                in_=idx_all[:, NBUF - 1 :, :].bitcast(i32),
            )
            nc.sync.dma_start(
                out=w_out_v[:, NBUF - 1 :, :], in_=w8_all[:, NBUF - 1 :, :]
            )
    nc.compile()
    return nc


def kernel(x, weight, bias):
    global LAST_RESULT
    x = np.asarray(x, dtype=np.float32)
    weight = np.asarray(weight, dtype=np.float32)
    bias = np.asarray(bias, dtype=np.float32).reshape(1, E)

    if "nc" not in _cache:
        _cache["nc"] = _build()
    nc = _cache["nc"]

    s_x = float(np.abs(x).max()) / 32767.0
    # x ships as x/s_x (int16 + int8/256); fold s_x into the weights so the
    # device-side reconstruction needs no scale operand
    wTh = np.ascontiguousarray(weight.T * np.float32(s_x))  # [D, E] f32

    def shard(c):
        xs = x[c * TOK_PC : (c + 1) * TOK_PC]  # [1024, D]
        xsc = xs.T / s_x  # [D, 1024]
        hi = np.rint(xsc)
        lo = np.clip(np.rint((xsc - hi) * 256.0), -127, 127).astype(np.int8)
        hi = hi.astype(np.int16)

        def pack(a):
            return np.ascontiguousarray(
                a.reshape(KC, 128, NBUF, XBUF_T)
                .transpose(2, 1, 0, 3)
                .reshape(NBUF * 128, KC * XBUF_T)
            )

        return pack(hi), pack(lo)

    with ThreadPoolExecutor(N_CORES) as ex:
        packed = list(ex.map(shard, range(N_CORES)))

    in_maps = [
        {
            "xh": packed[c][0],
            "xl": packed[c][1],
            "wT": wTh,
            "bias": bias,
        }
        for c in range(N_CORES)
    ]
    res = run_bass_kernel_spmd(nc, in_maps, list(range(N_CORES)))
    LAST_RESULT = res
    w = np.concatenate([res.results[c]["w_out"] for c in range(N_CORES)], axis=0)
    idx = np.concatenate([res.results[c]["idx_out"] for c in range(N_CORES)], axis=0)
    return w, idx.astype(np.int32)


# revision 45
# speedup vs baseline: 1.0325x; 1.0325x over previous
"""MoE gate routing kernel for Trainium2 (8 NeuronCores, data-parallel over tokens).

Computes, for x[8192,7168], weight[256,7168], bias[256]:
    scores = sigmoid(x @ weight.T + bias)            # [N, 256]
    group top-2 sums over 8 groups of 32 -> pick best group
    top-8 experts within best group (global indices), weights = renormalized
    sigmoid scores * 2.5
Returns (w [8192,8] f32, idx [8192,8] i32).

Strategy: shard tokens 8-way (1024/core). The kernel is HBM-bound: fp32 x is
29.4 MB/core, and cheaper encodings of x flip router top-k decisions past the
2e-2 gate (fp16: 2.27e-2; int16-only: 2.06e-2 -- both hardware-measured, the
idx rel-err metric is dominated by a handful of group-flip tokens). So x ships
as int16 + int8 residual (24-bit fixed point, reconstruction exact to ~1e-6 =
below fp32 accumulation noise) at 3 B/elem = 22 MB, and weight ships as raw
fp32 typed f32r (7.3 MB). Accuracy is then identical to the fp32r baseline
(1.35e-2) while DMA drops 20%.

On device the reconstruction x = i16*s + i8*(s/256) runs as two passes over
otherwise-idle engines: pass 1 on ScalarE (activation copy-with-scale), pass 2
on Pool (scalar_tensor_tensor fused multiply-add) writing f32r for the
full-rate fp32r matmul. VectorE keeps the top-k chain. Work is quartered so
reconstruction pipelines against the DMA stream.

x is host-packed as [NBUF, 128, KC, 128] so each half-buffer DMA is one
contiguous descriptor per partition (full DMA rate).
"""

import sys

sys.path.insert(0, "/opt/trn_rl_repo")

from concurrent.futures import ThreadPoolExecutor

import numpy as np

import concourse.bass as bass
from concourse import bacc
import concourse.mybir as mybir
from concourse.bass_utils import run_bass_kernel_spmd
from concourse.tile import TileContext

N_CORES = 8
N_TOK = 8192
TOK_PC = N_TOK // N_CORES  # 1024 tokens per core
D = 7168
E = 256
G = 8  # groups
EPG = E // G  # 32 experts per group
TOPK = 8
ROUTE_SCALE = 2.5
KC = D // 128  # 56 k-chunks
KCH = KC // 2  # 28 k-chunks per half-buffer
KQ = KC // 4  # 14 k-chunks per recon quarter
XBUF_T = 128  # tokens per x buffer / subtile
NBUF = TOK_PC // XBUF_T  # 8 buffers/subtiles per core

f32 = mybir.dt.float32
f32r = mybir.dt.float32r
i16 = mybir.dt.int16
i8 = mybir.dt.int8
i32 = mybir.dt.int32
u32 = mybir.dt.uint32
AX = mybir.AxisListType
OP = mybir.AluOpType
ACTF = mybir.ActivationFunctionType

_cache = {}
LAST_RESULT = None  # BassKernelResults of the most recent run (for test harness)


def _build():
    nc = bacc.Bacc(None, target_bir_lowering=False)

    # x packed [NBUF, 128, KC, XBUF_T] flattened to 2D, hi/lo streams
    xh = nc.declare_dram_parameter("xh", [NBUF * 128, KC * XBUF_T], i16, isOutput=False)
    xl = nc.declare_dram_parameter("xl", [NBUF * 128, KC * XBUF_T], i8, isOutput=False)
    wT = nc.declare_dram_parameter("wT", [D, E], f32, isOutput=False)
    bias = nc.declare_dram_parameter("bias", [1, E], f32, isOutput=False)
    w_out = nc.declare_dram_parameter("w_out", [TOK_PC, TOPK], f32, isOutput=True)
    idx_out = nc.declare_dram_parameter("idx_out", [TOK_PC, TOPK], i32, isOutput=True)

    xh_v = xh.rearrange("(b p) (c n) -> b p c n", p=128, c=KC)
    xl_v = xl.rearrange("(b p) (c n) -> b p c n", p=128, c=KC)
    wT_v = wT.rearrange("(c p) e -> p c e", p=128)  # [128, KC, E]

    with TileContext(nc) as tc:
        with (
            tc.tile_pool(name="const", bufs=1) as cpool,
            tc.tile_pool(name="xh", bufs=8) as hpool,
            tc.tile_pool(name="xl", bufs=8) as lpool,
            tc.tile_pool(name="xt", bufs=3) as tpool,
            tc.tile_pool(name="x32", bufs=8) as xpool,
            tc.tile_pool(name="sb", bufs=3) as spool,
            tc.tile_pool(name="small", bufs=3) as mpool,
            tc.tile_pool(name="out", bufs=3) as opool,
            tc.tile_pool(name="psum", bufs=6, space="PSUM") as ppool,
        ):
            def dma_xpiece(s, h):
                # two quarter-DMA pairs per half: recon can start ~2us after
                # the first quarter lands instead of ~4us after the half
                out = []
                for qq in range(2):
                    q = 2 * h + qq
                    ht = hpool.tile([128, KQ, XBUF_T], i16, tag="xh")
                    nc.sync.dma_start(
                        out=ht, in_=xh_v[s, :, q * KQ : (q + 1) * KQ, :]
                    )
                    lt = lpool.tile([128, KQ, XBUF_T], i8, tag="xl")
                    nc.sync.dma_start(
                        out=lt, in_=xl_v[s, :, q * KQ : (q + 1) * KQ, :]
                    )
                    out.append((ht, lt))
                return out

            # x and weight-quarter DMAs interleave so neither stream starves:
            # x0, w0, x1, w1, w2, x2, w3, then x3..x7
            w32 = cpool.tile([128, KC, E], f32r)

            def dma_wq(q):
                sl = slice(q * KQ, (q + 1) * KQ)
                nc.sync.dma_start(out=w32[:, sl, :], in_=wT_v[:, sl, :].bitcast(f32r))

            pre = {0: dma_xpiece(0, 0) + dma_xpiece(0, 1)}
            bias_sb = cpool.tile([1, E], f32)
            nc.sync.dma_start(out=bias_sb, in_=bias[:, :])
            ones_sb = cpool.tile([1, 128], f32)
            nc.vector.memset(ones_sb, 1.0)

            dma_wq(0)
            pre[1] = dma_xpiece(1, 0) + dma_xpiece(1, 1)
            dma_wq(1)
            pre[2] = dma_xpiece(2, 0) + dma_xpiece(2, 1)
            dma_wq(2)
            dma_wq(3)

            # accumulate outputs in SBUF; a single DMA pair at the very end
            # keeps the SP sequencer's DMA stream free of data-dependent
            # waits (a per-subtile output DMA would head-of-line-block the
            # later x-input DMAs behind it)
            w8_all = cpool.tile([128, NBUF, TOPK], f32)
            idx_all = cpool.tile([128, NBUF, TOPK], u32)

            def recon_quarter(ht, lt, q, on_pool):
                """x/s = i16 + i8/256 (the x scale is folded into the host-
                scaled weights). ACT casts the hi stream to f32; the lo stream
                fuses in via DVE scalar_tensor_tensor, except one quarter per
                buffer routed to Pool (tensor_scalar+tensor_add pair) to keep
                DVE off the critical cadence. Quarter-sized tiles so matmuls
                start as soon as each quarter is reconstructed."""
                xt = tpool.tile([128, KQ, XBUF_T], f32, tag="xt")
                nc.scalar.mul(xt, ht, 1.0)
                x32q = xpool.tile([128, KQ, XBUF_T], f32r, tag="x32")
                if on_pool:
                    lo = tpool.tile([128, KQ, XBUF_T], f32, tag="lo")
                    nc.gpsimd.tensor_scalar(lo, lt, 0.00390625, None, op0=OP.mult)
                    nc.gpsimd.tensor_add(x32q, lo, xt)
                else:
                    nc.vector.scalar_tensor_tensor(
                        out=x32q,
                        in0=lt,
                        scalar=0.00390625,
                        in1=xt,
                        op0=OP.mult,
                        op1=OP.add,
                    )
                return x32q

            for s in range(NBUF):
                if s in pre:
                    pieces = pre[s]
                else:
                    pieces = dma_xpiece(s, 0) + dma_xpiece(s, 1)

                t0 = s * XBUF_T
                ps = ppool.tile([128, E], f32, tag="ps")
                # bias preload: ps[t, e] = 1 * bias[e] (plain f32 matmul)
                nc.tensor.matmul(
                    out=ps, lhsT=ones_sb, rhs=bias_sb, start=True, stop=False
                )
                for qg in range(4):
                    ht, lt = pieces[qg]
                    on_pool = qg == 3
                    x32q = recon_quarter(ht, lt, qg, on_pool)
                    if True:
                        for c in range(KQ):
                            cc = qg * KQ + c
                            nc.tensor.matmul(
                                out=ps,
                                lhsT=x32q[:, c, :],
                                rhs=w32[:, cc, :],
                                start=False,
                                stop=(cc == KC - 1),
                            )

                sig = spool.tile([128, G, EPG], f32, tag="sig")
                nc.scalar.activation(
                    out=sig.rearrange("p g e -> p (g e)"), in_=ps, func=ACTF.Sigmoid
                )
                sig_flat = sig.rearrange("p g e -> p (g e)")

                # group top-2 sum
                m1 = mpool.tile([128, G], f32, tag="m1")
                nc.vector.tensor_reduce(out=m1, in_=sig, axis=AX.X, op=OP.max)
                scr = spool.tile([128, G, EPG], f32, tag="scr")
                nc.vector.match_replace(
                    out=scr.rearrange("p g e -> p (g e)"),
                    in_to_replace=m1,
                    in_values=sig_flat,
                    imm_value=-1e30,
                )
                gs = mpool.tile([128, G], f32, tag="gs")
                nc.vector.tensor_reduce(out=gs, in_=scr, axis=AX.X, op=OP.max)
                nc.vector.tensor_add(gs, gs, m1)  # m1 + m2

                # one-hot of best group -> multiplicative mask
                gmax = mpool.tile([128, 1], f32, tag="gmax")
                nc.vector.tensor_reduce(out=gmax, in_=gs, axis=AX.X, op=OP.max)
                eq = mpool.tile([128, G, 1], f32, tag="eq")
                nc.vector.tensor_scalar(eq[:, :, 0], gs, gmax, None, op0=OP.is_ge)
                # masked scores: kept group unchanged (x1.0), others -> 0.0
                masked = spool.tile([128, G, EPG], f32, tag="masked")
                ba, bb = bass.broadcast_tensor_aps(sig[:, :, :], eq[:, :, :])
                nc.vector.tensor_tensor(out=masked, in0=ba, in1=bb, op=OP.mult)
                masked_flat = masked.rearrange("p g e -> p (g e)")

                vals8 = mpool.tile([128, TOPK], f32, tag="vals8")
                nc.vector.max(out=vals8, in_=masked_flat)
                nc.vector.max_index(
                    out=idx_all[:, s, :], in_max=vals8, in_values=masked_flat
                )

                ssum = mpool.tile([128, 1], f32, tag="ssum")
                nc.vector.tensor_reduce(out=ssum, in_=vals8, axis=AX.X, op=OP.add)
                rcp = mpool.tile([128, 1], f32, tag="rcp")
                nc.vector.reciprocal(out=rcp, in_=ssum)
                nc.vector.tensor_scalar(
                    w8_all[:, s, :], vals8, rcp, ROUTE_SCALE, op0=OP.mult, op1=OP.mult
                )

            w_out_v = w_out.rearrange("(s p) k -> p s k", p=128)
            idx_out_v = idx_out.rearrange("(s p) k -> p s k", p=128)
            nc.sync.dma_start(
                out=idx_out_v[:, : NBUF - 1, :],
                in_=idx_all[:, : NBUF - 1, :].bitcast(i32),
            )
            nc.sync.dma_start(
                out=w_out_v[:, : NBUF - 1, :], in_=w8_all[:, : NBUF - 1, :]
            )
            nc.sync.dma_start(
                out=idx_out_v[:, NBUF - 1 :, :],
                in_=idx_all[:, NBUF - 1 :, :].bitcast(i32),
            )
            nc.sync.dma_start(
                out=w_out_v[:, NBUF - 1 :, :], in_=w8_all[:, NBUF - 1 :, :]
            )
    nc.compile()
    return nc


def kernel(x, weight, bias):
    global LAST_RESULT
    x = np.asarray(x, dtype=np.float32)
    weight = np.asarray(weight, dtype=np.float32)
    bias = np.asarray(bias, dtype=np.float32).reshape(1, E)

    if "nc" not in _cache:
        _cache["nc"] = _build()
    nc = _cache["nc"]

    s_x = float(np.abs(x).max()) / 32767.0
    # x ships as x/s_x (int16 + int8/256); fold s_x into the weights so the
    # device-side reconstruction needs no scale operand
    wTh = np.ascontiguousarray(weight.T * np.float32(s_x))  # [D, E] f32

    def shard(c):
        xs = x[c * TOK_PC : (c + 1) * TOK_PC]  # [1024, D]
        xsc = xs.T / s_x  # [D, 1024]
        hi = np.rint(xsc)
        lo = np.clip(np.rint((xsc - hi) * 256.0), -127, 127).astype(np.int8)
        hi = hi.astype(np.int16)

        def pack(a):
            return np.ascontiguousarray(
                a.reshape(KC, 128, NBUF, XBUF_T)
                .transpose(2, 1, 0, 3)
                .reshape(NBUF * 128, KC * XBUF_T)
            )

        return pack(hi), pack(lo)

    with ThreadPoolExecutor(N_CORES) as ex:
        packed = list(ex.map(shard, range(N_CORES)))

    in_maps = [
        {
            "xh": packed[c][0],
            "xl": packed[c][1],
            "wT": wTh,
            "bias": bias,
        }
        for c in range(N_CORES)
    ]
    res = run_bass_kernel_spmd(nc, in_maps, list(range(N_CORES)))
    LAST_RESULT = res
    w = np.concatenate([res.results[c]["w_out"] for c in range(N_CORES)], axis=0)
    idx = np.concatenate([res.results[c]["idx_out"] for c in range(N_CORES)], axis=0)
    return w, idx.astype(np.int32)


# revision 48
# speedup vs baseline: 1.0513x; 1.0182x over previous
"""MoE gate routing kernel for Trainium2 (8 NeuronCores, data-parallel over tokens).

Computes, for x[8192,7168], weight[256,7168], bias[256]:
    scores = sigmoid(x @ weight.T + bias)            # [N, 256]
    group top-2 sums over 8 groups of 32 -> pick best group
    top-8 experts within best group (global indices), weights = renormalized
    sigmoid scores * 2.5
Returns (w [8192,8] f32, idx [8192,8] i32).

Strategy: shard tokens 8-way (1024/core). The kernel is HBM-bound: fp32 x is
29.4 MB/core, and cheaper encodings of x flip router top-k decisions past the
2e-2 gate (fp16: 2.27e-2; int16-only: 2.06e-2 -- both hardware-measured, the
idx rel-err metric is dominated by a handful of group-flip tokens). So x ships
as int16 + int8 residual (24-bit fixed point, reconstruction exact to ~1e-6 =
below fp32 accumulation noise) at 3 B/elem = 22 MB, and weight ships as raw
fp32 typed f32r (7.3 MB). Accuracy is then identical to the fp32r baseline
(1.35e-2) while DMA drops 20%.

On device the reconstruction x = i16*s + i8*(s/256) runs as two passes over
otherwise-idle engines: pass 1 on ScalarE (activation copy-with-scale), pass 2
on Pool (scalar_tensor_tensor fused multiply-add) writing f32r for the
full-rate fp32r matmul. VectorE keeps the top-k chain. Work is quartered so
reconstruction pipelines against the DMA stream.

x is host-packed as [NBUF, 128, KC, 128] so each half-buffer DMA is one
contiguous descriptor per partition (full DMA rate).
"""

import sys

sys.path.insert(0, "/opt/trn_rl_repo")

from concurrent.futures import ThreadPoolExecutor

import numpy as np

import concourse.bass as bass
from concourse import bacc
import concourse.mybir as mybir
from concourse.bass_utils import run_bass_kernel_spmd
from concourse.tile import TileContext

N_CORES = 8
N_TOK = 8192
TOK_PC = N_TOK // N_CORES  # 1024 tokens per core
D = 7168
E = 256
G = 8  # groups
EPG = E // G  # 32 experts per group
TOPK = 8
ROUTE_SCALE = 2.5
KC = D // 128  # 56 k-chunks
KCH = KC // 2  # 28 k-chunks per half-buffer
KQ = KC // 4  # 14 k-chunks per recon quarter
XBUF_T = 128  # tokens per x buffer / subtile
NBUF = TOK_PC // XBUF_T  # 8 buffers/subtiles per core

f32 = mybir.dt.float32
f32r = mybir.dt.float32r
i16 = mybir.dt.int16
i8 = mybir.dt.int8
i32 = mybir.dt.int32
u32 = mybir.dt.uint32
AX = mybir.AxisListType
OP = mybir.AluOpType
ACTF = mybir.ActivationFunctionType

_cache = {}
LAST_RESULT = None  # BassKernelResults of the most recent run (for test harness)


def _build():
    nc = bacc.Bacc(None, target_bir_lowering=False)

    # x packed [NBUF, 128, KC, XBUF_T] flattened to 2D, hi/lo streams
    xh = nc.declare_dram_parameter("xh", [NBUF * 128, KC * XBUF_T], i16, isOutput=False)
    xl = nc.declare_dram_parameter("xl", [NBUF * 128, KC * XBUF_T], i8, isOutput=False)
    wT = nc.declare_dram_parameter("wT", [D, E], f32, isOutput=False)
    bias = nc.declare_dram_parameter("bias", [1, E], f32, isOutput=False)
    w_out = nc.declare_dram_parameter("w_out", [TOK_PC, TOPK], f32, isOutput=True)
    idx_out = nc.declare_dram_parameter("idx_out", [TOK_PC, TOPK], i32, isOutput=True)

    xh_v = xh.rearrange("(b p) (c n) -> b p c n", p=128, c=KC)
    xl_v = xl.rearrange("(b p) (c n) -> b p c n", p=128, c=KC)
    wT_v = wT.rearrange("(c p) e -> p c e", p=128)  # [128, KC, E]

    with TileContext(nc) as tc:
        with (
            tc.tile_pool(name="const", bufs=1) as cpool,
            tc.tile_pool(name="xh", bufs=8) as hpool,
            tc.tile_pool(name="xl", bufs=8) as lpool,
            tc.tile_pool(name="xt", bufs=4) as tpool,
            tc.tile_pool(name="x32", bufs=6) as xpool,
            tc.tile_pool(name="sb", bufs=2) as spool,
            tc.tile_pool(name="small", bufs=3) as mpool,
            tc.tile_pool(name="out", bufs=3) as opool,
            tc.tile_pool(name="psum", bufs=6, space="PSUM") as ppool,
        ):
            def dma_xpiece(s, h):
                # two quarter-DMA pairs per half: recon can start ~2us after
                # the first quarter lands instead of ~4us after the half
                out = []
                for qq in range(2):
                    q = 2 * h + qq
                    ht = hpool.tile([128, KQ, XBUF_T], i16, tag="xh")
                    nc.sync.dma_start(
                        out=ht, in_=xh_v[s, :, q * KQ : (q + 1) * KQ, :]
                    )
                    lt = lpool.tile([128, KQ, XBUF_T], i8, tag="xl")
                    nc.sync.dma_start(
                        out=lt, in_=xl_v[s, :, q * KQ : (q + 1) * KQ, :]
                    )
                    out.append((ht, lt))
                return out

            # x and weight-quarter DMAs interleave so neither stream starves:
            # x0, w0, x1, w1, w2, x2, w3, then x3..x7
            w32 = cpool.tile([128, KC, E], f32r)

            def dma_wq(q):
                sl = slice(q * KQ, (q + 1) * KQ)
                nc.sync.dma_start(out=w32[:, sl, :], in_=wT_v[:, sl, :].bitcast(f32r))

            pre = {0: dma_xpiece(0, 0) + dma_xpiece(0, 1)}
            bias_sb = cpool.tile([1, E], f32)
            nc.sync.dma_start(out=bias_sb, in_=bias[:, :])
            ones_sb = cpool.tile([1, 128], f32)
            nc.vector.memset(ones_sb, 1.0)

            dma_wq(0)
            pre[1] = dma_xpiece(1, 0) + dma_xpiece(1, 1)
            dma_wq(1)
            pre[2] = dma_xpiece(2, 0) + dma_xpiece(2, 1)
            dma_wq(2)
            dma_wq(3)

            # accumulate outputs in SBUF; a single DMA pair at the very end
            # keeps the SP sequencer's DMA stream free of data-dependent
            # waits (a per-subtile output DMA would head-of-line-block the
            # later x-input DMAs behind it)
            w8_all = cpool.tile([128, NBUF, TOPK], f32)
            idx_all = cpool.tile([128, NBUF, TOPK], u32)

            def recon_quarter(ht, lt, q, on_pool):
                """x/s = i16 + i8/256 (the x scale is folded into the host-
                scaled weights). ACT casts the hi stream to f32; the lo stream
                fuses in via DVE scalar_tensor_tensor, except one quarter per
                buffer routed to Pool (tensor_scalar+tensor_add pair) to keep
                DVE off the critical cadence. Quarter-sized tiles so matmuls
                start as soon as each quarter is reconstructed."""
                xt = tpool.tile([128, KQ, XBUF_T], f32, tag="xt")
                nc.scalar.mul(xt, ht, 1.0)
                x32q = xpool.tile([128, KQ, XBUF_T], f32r, tag="x32")
                if on_pool:
                    lo = tpool.tile([128, KQ, XBUF_T], f32, tag="lo")
                    nc.gpsimd.tensor_scalar(lo, lt, 0.00390625, None, op0=OP.mult)
                    nc.gpsimd.tensor_add(x32q, lo, xt)
                else:
                    nc.vector.scalar_tensor_tensor(
                        out=x32q,
                        in0=lt,
                        scalar=0.00390625,
                        in1=xt,
                        op0=OP.mult,
                        op1=OP.add,
                    )
                return x32q

            for s in range(NBUF):
                if s in pre:
                    pieces = pre[s]
                else:
                    pieces = dma_xpiece(s, 0) + dma_xpiece(s, 1)

                t0 = s * XBUF_T
                ps = ppool.tile([128, E], f32, tag="ps")
                # bias preload: ps[t, e] = 1 * bias[e] (plain f32 matmul)
                nc.tensor.matmul(
                    out=ps, lhsT=ones_sb, rhs=bias_sb, start=True, stop=False
                )
                for qg in range(4):
                    ht, lt = pieces[qg]
                    on_pool = qg == 3
                    x32q = recon_quarter(ht, lt, qg, on_pool)
                    if True:
                        for c in range(KQ):
                            cc = qg * KQ + c
                            nc.tensor.matmul(
                                out=ps,
                                lhsT=x32q[:, c, :],
                                rhs=w32[:, cc, :],
                                start=False,
                                stop=(cc == KC - 1),
                            )

                sig = spool.tile([128, G, EPG], f32, tag="sig")
                nc.scalar.activation(
                    out=sig.rearrange("p g e -> p (g e)"), in_=ps, func=ACTF.Sigmoid
                )
                sig_flat = sig.rearrange("p g e -> p (g e)")

                # group top-2 sum
                m1 = mpool.tile([128, G], f32, tag="m1")
                nc.vector.tensor_reduce(out=m1, in_=sig, axis=AX.X, op=OP.max)
                scr = spool.tile([128, G, EPG], f32, tag="scr")
                nc.vector.match_replace(
                    out=scr.rearrange("p g e -> p (g e)"),
                    in_to_replace=m1,
                    in_values=sig_flat,
                    imm_value=-1e30,
                )
                gs = mpool.tile([128, G], f32, tag="gs")
                nc.vector.tensor_reduce(out=gs, in_=scr, axis=AX.X, op=OP.max)
                nc.vector.tensor_add(gs, gs, m1)  # m1 + m2

                # one-hot of best group -> multiplicative mask
                gmax = mpool.tile([128, 1], f32, tag="gmax")
                nc.vector.tensor_reduce(out=gmax, in_=gs, axis=AX.X, op=OP.max)
                eq = mpool.tile([128, G, 1], f32, tag="eq")
                nc.vector.tensor_scalar(eq[:, :, 0], gs, gmax, None, op0=OP.is_ge)
                # masked scores: kept group unchanged (x1.0), others -> 0.0
                masked = spool.tile([128, G, EPG], f32, tag="masked")
                ba, bb = bass.broadcast_tensor_aps(sig[:, :, :], eq[:, :, :])
                nc.vector.tensor_tensor(out=masked, in0=ba, in1=bb, op=OP.mult)
                masked_flat = masked.rearrange("p g e -> p (g e)")

                vals8 = mpool.tile([128, TOPK], f32, tag="vals8")
                nc.vector.max(out=vals8, in_=masked_flat)
                nc.vector.max_index(
                    out=idx_all[:, s, :], in_max=vals8, in_values=masked_flat
                )

                ssum = mpool.tile([128, 1], f32, tag="ssum")
                nc.vector.tensor_reduce(out=ssum, in_=vals8, axis=AX.X, op=OP.add)
                rcp = mpool.tile([128, 1], f32, tag="rcp")
                nc.vector.reciprocal(out=rcp, in_=ssum)
                nc.vector.tensor_scalar(
                    w8_all[:, s, :], vals8, rcp, ROUTE_SCALE, op0=OP.mult, op1=OP.mult
                )

            w_out_v = w_out.rearrange("(s p) k -> p s k", p=128)
            idx_out_v = idx_out.rearrange("(s p) k -> p s k", p=128)
            nc.sync.dma_start(
                out=idx_out_v[:, : NBUF - 1, :],
                in_=idx_all[:, : NBUF - 1, :].bitcast(i32),
            )
            nc.sync.dma_start(
                out=w_out_v[:, : NBUF - 1, :], in_=w8_all[:, : NBUF - 1, :]
            )
            nc.sync.dma_start(
                out=idx_out_v[:, NBUF - 1 :, :],
                in_=idx_all[:, NBUF - 1 :, :].bitcast(i32),
            )
            nc.sync.dma_start(
                out=w_out_v[:, NBUF - 1 :, :], in_=w8_all[:, NBUF - 1 :, :]
            )
    nc.compile()
    return nc


def kernel(x, weight, bias):
    global LAST_RESULT
    x = np.asarray(x, dtype=np.float32)
    weight = np.asarray(weight, dtype=np.float32)
    bias = np.asarray(bias, dtype=np.float32).reshape(1, E)

    if "nc" not in _cache:
        _cache["nc"] = _build()
    nc = _cache["nc"]

    s_x = float(np.abs(x).max()) / 32767.0
    # x ships as x/s_x (int16 + int8/256); fold s_x into the weights so the
    # device-side reconstruction needs no scale operand
    wTh = np.ascontiguousarray(weight.T * np.float32(s_x))  # [D, E] f32

    def shard(c):
        xs = x[c * TOK_PC : (c + 1) * TOK_PC]  # [1024, D]
        xsc = xs.T / s_x  # [D, 1024]
        hi = np.rint(xsc)
        lo = np.clip(np.rint((xsc - hi) * 256.0), -127, 127).astype(np.int8)
        hi = hi.astype(np.int16)

        def pack(a):
            return np.ascontiguousarray(
                a.reshape(KC, 128, NBUF, XBUF_T)
                .transpose(2, 1, 0, 3)
                .reshape(NBUF * 128, KC * XBUF_T)
            )

        return pack(hi), pack(lo)

    with ThreadPoolExecutor(N_CORES) as ex:
        packed = list(ex.map(shard, range(N_CORES)))

    in_maps = [
        {
            "xh": packed[c][0],
            "xl": packed[c][1],
            "wT": wTh,
            "bias": bias,
        }
        for c in range(N_CORES)
    ]
    res = run_bass_kernel_spmd(nc, in_maps, list(range(N_CORES)))
    LAST_RESULT = res
    w = np.concatenate([res.results[c]["w_out"] for c in range(N_CORES)], axis=0)
    idx = np.concatenate([res.results[c]["idx_out"] for c in range(N_CORES)], axis=0)
    return w, idx.astype(np.int32)
